# revision 30
# baseline (speedup 1.0000x reference)
# Trainium2 Bass kernel for GPT-J-style cosine attention (no softmax).
#
# Reference computation (B=2, S=1024, E=2048, H=16, HD=128, ROT=64):
#   q/k/v = hs @ W.T ; partial rotary on first 64 dims of each head;
#   v /= max(count^sigmoid(norm_const), 1); q,k L2-normalized; q,k,v
#   masked by attention_mask==0 rows; attn = tril(q @ k.T) (zeros, no
#   softmax); out = (attn @ v) @ w_o.T.
#
# Sharding: core c = b*4 + g  (b in 0..1 batch, g in 0..3 head-group of
# 4 heads). Each core computes its batch's S x 512 slice of q/k/v, runs
# attention for its 4 heads, and produces a partial [S, E] out-proj
# contribution; the host sums the 4 partials per batch.
import numpy as np

B, S, E, H, HD, ROT, MAXP = 2, 1024, 2048, 16, 128, 64, 2048
HL = 4            # heads per core
GD = HL * HD      # 512 output dims per core
NB = S // 128     # 8 s-blocks
NK = E // 128     # 16 contraction tiles
EPS = 1e-12


def _sinusoidal(num_pos, dim):
    inv_freq = 1.0 / (10000.0 ** (np.arange(0, dim, 2, dtype=np.float32) / dim))
    sinusoid = np.einsum("i,j->ij", np.arange(num_pos, dtype=np.float32), inv_freq)
    return np.concatenate([np.sin(sinusoid), np.cos(sinusoid)], axis=-1)


_BUILT = None


def _build():
    global _BUILT
    if _BUILT is not None:
        return _BUILT
    import concourse.bacc as bacc
    import concourse.mybir as mybir
    from concourse.tile import TileContext

    F32 = mybir.dt.float32
    F32R = mybir.dt.float32r
    BF16 = mybir.dt.bfloat16
    MUL = mybir.AluOpType.mult
    SQUARE = mybir.ActivationFunctionType.Square

    nc = bacc.Bacc(None, target_bir_lowering=False)

    hsT = nc.dram_tensor("hsT", [NB, 128, NK * 128], BF16, kind="ExternalInput")
    wqT = nc.dram_tensor("wqT", [E, GD], BF16, kind="ExternalInput")
    wkT = nc.dram_tensor("wkT", [E, GD], BF16, kind="ExternalInput")
    wvT = nc.dram_tensor("wvT", [E, GD], BF16, kind="ExternalInput")
    woT = nc.dram_tensor("woT", [GD, E], BF16, kind="ExternalInput")
    cos4d = nc.dram_tensor("cos4", [128, NB, HL, ROT], BF16, kind="ExternalInput")
    sin4d = nc.dram_tensor("sin4", [128, NB, HL, ROT], BF16, kind="ExternalInput")
    masksd = nc.dram_tensor("masks", [128, 4, 512], F32R, kind="ExternalInput")
    vscaled = nc.dram_tensor("vscale", [128, NB, HL], F32, kind="ExternalInput")
    qmaskd = nc.dram_tensor("qmask", [128, NB], F32, kind="ExternalInput")
    identd = nc.dram_tensor("ident", [128, 128], BF16, kind="ExternalInput")
    outd = nc.dram_tensor("out", [S, E], BF16, kind="ExternalOutput")

    import concourse.bass as bass
    with TileContext(nc) as tc:
        from contextlib import ExitStack
        ctx = ExitStack()
        with ctx:
            const = ctx.enter_context(tc.tile_pool(name="const", bufs=1))
            qkT_pool = ctx.enter_context(tc.tile_pool(name="qkT", bufs=1))
            vn_pool = ctx.enter_context(tc.tile_pool(name="vn", bufs=1))
            scr = ctx.enter_context(tc.tile_pool(name="scr", bufs=4))
            rot_pool = ctx.enter_context(tc.tile_pool(name="rot", bufs=5))
            ps_proj = ctx.enter_context(tc.tile_pool(name="ps_proj", bufs=3, space="PSUM"))
            ps_tr = ctx.enter_context(tc.tile_pool(name="ps_tr", bufs=1, space="PSUM"))
            ps_at = ctx.enter_context(tc.tile_pool(name="ps_at", bufs=2, space="PSUM"))
            ps_ao = ctx.enter_context(tc.tile_pool(name="ps_ao", bufs=2, space="PSUM"))

            cos4 = const.tile([128, NB, HL, ROT], BF16)
            sin4 = const.tile([128, NB, HL, ROT], BF16)
            masks = const.tile([128, 4, 512], F32R)
            vscale = const.tile([128, NB, HL], F32)
            qmask = const.tile([128, NB], F32)
            ident = const.tile([128, 128], BF16)
            nc.scalar.dma_start(out=ident[:], in_=identd[:])
            nc.scalar.dma_start(out=qmask[:], in_=qmaskd[:])
            nc.scalar.dma_start(out=cos4[:], in_=cos4d[:])
            nc.scalar.dma_start(out=sin4[:], in_=sin4d[:])
            nc.scalar.dma_start(out=vscale[:], in_=vscaled[:])
            nc.scalar.dma_start(out=masks[:], in_=masksd[:])

            # HAM warmup: keep PE busy on dummy matmuls over the ident tile
            # (first DMA to land) so the clock gate opens to 2.4 GHz before
            # the DMA-paced Q-projection starts issuing real matmuls. Sized
            # to bridge the whole wq+hs load window without a >3us PE idle.
            warm_ps = ps_tr.tile([128, 128], F32, tag="pt")
            for _ in range(64):
                nc.tensor.matmul(warm_ps[:], ident[:], ident[:],
                                 start=True, stop=True)

            # persistent transposed q/k: per local head, [hd=128, S]
            qT = [qkT_pool.tile([128, S], BF16, name=f"qT{h}") for h in range(HL)]
            kT = [qkT_pool.tile([128, S], BF16, name=f"kT{h}") for h in range(HL)]
            # v in natural layout per s-block: [128, 512]
            vn = [vn_pool.tile([128, GD], BF16, name=f"vn{m}") for m in range(NB)]

            with tc.tile_pool(name="hs", bufs=1) as hs_pool, \
                 tc.tile_pool(name="w", bufs=3) as w_pool:
                # hs blocked per s-block m: [128 E-sub, m, k, 128 s]
                hs = hs_pool.tile([128, NB, NK, 128], BF16)

                # one batched DMA per weight matrix (DMA *issues* cost
                # ~700ns each on the issuing engine - batch aggressively)
                def load_w(wd):
                    wt = w_pool.tile([128, NK, GD], BF16, tag="w")
                    nc.sync.dma_start(
                        out=wt[:],
                        in_=bass.AP(wd, 0, [[GD, 128], [128 * GD, NK], [1, GD]]))
                    return [wt[:, k] for k in range(NK)]

                # wq first (m=0 needs all of it); hs in halves on the
                # gpsimd queue so its transfers overlap the wq transfer
                wq = load_w(wqT)
                for half in range(2):
                    nc.gpsimd.dma_start(
                        out=hs[:, half * 4:(half + 1) * 4],
                        in_=bass.AP(hsT, half * 4 * 128 * NK * 128,
                                    [[NK * 128, 128], [128 * NK * 128, 4],
                                     [1, NK * 128]]))

                def proj_mms(wtiles, m):
                    ps = ps_proj.tile([128, GD], F32)
                    for k in range(NK):
                        nc.tensor.matmul(
                            ps[:], hs[:, m, k],
                            wtiles[k], start=(k == 0), stop=(k == NK - 1))
                    return ps

                def qk_postproc(ps, m):
                    # sum-of-squares per head (rotary is norm-preserving, so
                    # norms can be computed pre-rotary, straight from PSUM)
                    ss = scr.tile([128, HL], F32, tag="ss")
                    sqs = scr.tile([128, 128], F32, tag="sqs", bufs=1)
                    for h in range(HL):
                        nc.scalar.activation(out=sqs[:],
                                             in_=ps[:, h * 128:(h + 1) * 128],
                                             func=SQUARE, accum_out=ss[:, h:h + 1])
                    nrm = scr.tile([128, HL], F32, tag="nrm")
                    nc.scalar.sqrt(nrm[:], ss[:])
                    nc.vector.tensor_scalar_max(nrm[:], nrm[:], EPS)
                    rr = scr.tile([128, HL], F32, tag="rr")
                    nc.vector.reciprocal(rr[:], nrm[:])
                    nc.vector.tensor_scalar_mul(rr[:], rr[:], qmask[:, m:m + 1])
                    # evict PSUM -> SBUF with the per-row scale folded in
                    qn = rot_pool.tile([128, HL, 128], BF16, tag="qn")
                    for h in range(HL):
                        nc.vector.tensor_scalar_mul(qn[:, h], ps[:, h * 128:(h + 1) * 128],
                                                    rr[:, h:h + 1])
                    # GPT-J interleaved rotary on first ROT dims of each head
                    qrot = rot_pool.tile([128, HL, ROT], BF16, tag="qrot", bufs=2)
                    tmp2 = rot_pool.tile([128, HL, ROT], BF16, tag="tmp2", bufs=2)
                    nc.gpsimd.tensor_tensor(out=qrot[:, :, 0:ROT:2], in0=qn[:, :, 1:ROT:2],
                                            in1=sin4[:, m, :, 0:ROT:2], op=MUL)
                    nc.gpsimd.tensor_tensor(out=qrot[:, :, 1:ROT:2], in0=qn[:, :, 0:ROT:2],
                                            in1=sin4[:, m, :, 1:ROT:2], op=MUL)
                    nc.gpsimd.tensor_tensor(out=tmp2[:], in0=qn[:, :, 0:ROT],
                                            in1=cos4[:, m], op=MUL)
                    nc.gpsimd.tensor_add(out=qn[:, :, 0:ROT], in0=qrot[:], in1=tmp2[:])
                    return qn

                def transpose_block(qn, m, dstT):
                    for h in range(HL):
                        pt = ps_tr.tile([128, 128], BF16)
                        nc.tensor.transpose(pt[:], qn[:, h], ident[:])
                        nc.vector.tensor_copy(dstT[h][:, m * 128:(m + 1) * 128], pt[:])

                LAG = 3
                qns = {}
                for m in range(NB):
                    qns[m] = qk_postproc(proj_mms(wq, m), m)
                    if m >= LAG:
                        transpose_block(qns.pop(m - LAG), m - LAG, qT)
                wk = load_w(wkT)
                for m in range(NB - LAG, NB):
                    transpose_block(qns.pop(m), m, qT)
                for m in range(NB):
                    qns[m] = qk_postproc(proj_mms(wk, m), m)
                    if m >= LAG:
                        transpose_block(qns.pop(m - LAG), m - LAG, kT)
                wv = load_w(wvT)
                for m in range(NB - LAG, NB):
                    transpose_block(qns.pop(m), m, kT)
                for m in range(NB):
                    ps = proj_mms(wv, m)
                    for h in range(HL):
                        nc.vector.tensor_scalar_mul(vn[m][:, h * 128:(h + 1) * 128],
                                                    ps[:, h * 128:(h + 1) * 128],
                                                    vscale[:, m, h:h + 1])

            with tc.tile_pool(name="atn", bufs=14) as atn_pool, \
                 tc.tile_pool(name="aT", bufs=1) as aT_pool, \
                 tc.tile_pool(name="wo", bufs=1) as wo_pool, \
                 tc.tile_pool(name="ost", bufs=3) as ost_pool:
                aT = [aT_pool.tile([128, S], BF16, name=f"aT{h}") for h in range(HL)]

                # single batched DMA: wo_all[p, kk, :] = woT[kk*128+p, :]
                wo_all = wo_pool.tile([128, 4, E], BF16, tag="wo")
                nc.scalar.dma_start(
                    out=wo_all[:],
                    in_=bass.AP(woT, 0, [[E, 128], [128 * E, 4], [1, E]]))
                wo_tiles = [[wo_all[:, kk, n * 512:(n + 1) * 512]
                             for kk in range(4)] for n in range(4)]

                GE = mybir.AluOpType.is_ge
                diag_rr = 0
                for c in range(2):
                    for h in range(HL):
                        nblk = 4 * (c + 1)
                        at_tiles = []
                        for j in range(nblk):
                            pa = ps_at.tile([128, 512], F32)
                            nc.tensor.matmul(pa[:], kT[h][:, j * 128:(j + 1) * 128],
                                             qT[h][:, c * 512:(c + 1) * 512],
                                             start=True, stop=True)
                            at = atn_pool.tile([128, 512], BF16, tag="at")
                            jj = j - c * 4
                            if jj >= 0:  # diagonal block: apply causal mask
                                nc.vector.tensor_tensor(out=at[:], in0=pa[:],
                                                        in1=masks[:, jj], op=MUL)
                            else:        # fully below the diagonal
                                nc.scalar.copy(at[:], pa[:])
                            at_tiles.append(at)
                        po = ps_ao.tile([128, 512], F32)
                        for j in range(nblk):
                            nc.tensor.matmul(po[:], vn[j][:, h * 128:(h + 1) * 128],
                                             at_tiles[j][:],
                                             start=(j == 0), stop=(j == nblk - 1))
                        nc.scalar.copy(aT[h][:, c * 512:(c + 1) * 512], po[:])
                    # out-proj for this query-half overlaps next half's attn
                    for m in range(c * 4, (c + 1) * 4):
                        ot = ost_pool.tile([128, E], BF16, tag="ot")
                        for n in range(4):
                            ps = ps_proj.tile([128, 512], F32, tag="ps")
                            for k in range(HL):
                                nc.tensor.matmul(ps[:], aT[k][:, m * 128:(m + 1) * 128],
                                                 wo_tiles[n][k],
                                                 start=(k == 0), stop=(k == HL - 1))
                            # Vector carries the diag-mask evicts; bias the
                            # out-proj evictions toward Scalar to balance
                            if (4 * m + n) % 8 < 3:
                                nc.vector.tensor_copy(ot[:, n * 512:(n + 1) * 512], ps[:])
                            else:
                                nc.scalar.copy(ot[:, n * 512:(n + 1) * 512], ps[:])
                        nc.sync.dma_start(out=outd[m * 128:(m + 1) * 128, :], in_=ot[:])

    nc.compile()
    _BUILT = nc
    return nc


def _prep_inputs(hidden_states, w_q, w_k, w_v, w_o, norm_const,
                 attention_mask, position_ids):
    """Host-side shard + table prep. Returns list of 8 in_maps."""
    import ml_dtypes
    BF = ml_dtypes.bfloat16
    hidden_states = np.asarray(hidden_states, dtype=np.float32)
    w_q = np.asarray(w_q, dtype=np.float32)
    w_k = np.asarray(w_k, dtype=np.float32)
    w_v = np.asarray(w_v, dtype=np.float32)
    w_o = np.asarray(w_o, dtype=np.float32)
    norm_const = np.asarray(norm_const, dtype=np.float32).reshape(H)
    attention_mask = np.asarray(attention_mask, dtype=np.float32).reshape(B, S)
    position_ids = np.asarray(position_ids).reshape(B, S).astype(np.int64)

    embed = _sinusoidal(MAXP, ROT)                       # [MAXP, 64]
    sig = 1.0 / (1.0 + np.exp(-norm_const.astype(np.float64)))   # [H]
    mask0 = (attention_mask == 0).astype(np.float32)     # [B, S]
    counts = np.cumsum(mask0, axis=1).astype(np.float32)  # [B, S]
    denom = np.maximum(counts[:, None, :] ** sig[None, :, None], 1.0).astype(np.float32)
    vs_full = mask0[:, None, :] / denom                  # [B, H, S]

    # causal masks for the 4 diagonal-band block offsets
    p = np.arange(128)[:, None]
    f = np.arange(512)[None, :]
    masks = np.stack([(jj * 128 + p <= f) for jj in range(4)]).astype(np.float32)
    masks = np.ascontiguousarray(masks.transpose(1, 0, 2))  # [128, 4, 512]
    ident = np.eye(128, dtype=BF)

    in_maps = []
    for b in range(B):
        sincos = embed[position_ids[b]]                  # [S, 64]
        sin, cos = sincos[:, :ROT // 2], sincos[:, ROT // 2:]
        cosR = np.repeat(cos, 2, axis=1)                 # [S, 64]
        sinS = np.empty((S, ROT), dtype=np.float32)
        sinS[:, 0::2] = -sin
        sinS[:, 1::2] = sin
        # [S,64] -> [128 part, NB, 64] -> broadcast over HL heads
        def to4(t):
            t = t.reshape(NB, 128, ROT).transpose(1, 0, 2)
            return np.ascontiguousarray(np.broadcast_to(
                t[:, :, None, :], (128, NB, HL, ROT)).astype(BF))
        cos4 = to4(cosR)
        sin4 = to4(sinS)
        qm = np.ascontiguousarray(mask0[b].reshape(NB, 128).T)  # [128, NB]
        # [NB, 128 E-sub, NK*128]: hsT_b[m, p, k*128+c] = hs[b, m*128+c, k*128+p]
        hsT_b = np.ascontiguousarray(
            hidden_states[b].reshape(NB, 128, NK, 128)
            .transpose(0, 3, 2, 1).astype(BF)).reshape(NB, 128, NK * 128)
        for g in range(4):
            sl = slice(g * GD, (g + 1) * GD)
            vs = vs_full[b, 4 * g:4 * g + HL, :]                # [HL, S]
            vs = np.ascontiguousarray(
                vs.reshape(HL, NB, 128).transpose(2, 1, 0))     # [128, NB, HL]
            in_maps.append({
                "hsT": hsT_b,
                "wqT": np.ascontiguousarray(w_q[sl, :].T.astype(BF)),
                "wkT": np.ascontiguousarray(w_k[sl, :].T.astype(BF)),
                "wvT": np.ascontiguousarray(w_v[sl, :].T.astype(BF)),
                "woT": np.ascontiguousarray(w_o[:, sl].T.astype(BF)),
                "cos4": cos4, "sin4": sin4, "masks": masks,
                "vscale": vs, "qmask": qm, "ident": ident,
            })
    # core order: c = b*4 + g
    return in_maps


def run(inputs, trace=False, trace_cores=None):
    from concourse.bass_utils import run_bass_kernel_spmd
    nc = _build()
    in_maps = _prep_inputs(**inputs)
    res = run_bass_kernel_spmd(nc, in_maps, core_ids=list(range(8)),
                               trace=trace, trace_cores=trace_cores)
    partials = [np.asarray(res.results[c]["out"], dtype=np.float32)
                for c in range(8)]
    out = np.empty((B, S, E), dtype=np.float32)
    for b in range(B):
        out[b] = partials[4 * b] + partials[4 * b + 1] \
            + partials[4 * b + 2] + partials[4 * b + 3]
    return out, res


def kernel(**inputs):
    out, _ = run(inputs, trace=False)
    return out



# revision 33
# speedup vs baseline: 1.0117x; 1.0117x over previous
# Trainium2 Bass kernel for GPT-J-style cosine attention (no softmax).
#
# Reference computation (B=2, S=1024, E=2048, H=16, HD=128, ROT=64):
#   q/k/v = hs @ W.T ; partial rotary on first 64 dims of each head;
#   v /= max(count^sigmoid(norm_const), 1); q,k L2-normalized; q,k,v
#   masked by attention_mask==0 rows; attn = tril(q @ k.T) (zeros, no
#   softmax); out = (attn @ v) @ w_o.T.
#
# Sharding: core c = b*4 + g  (b in 0..1 batch, g in 0..3 head-group of
# 4 heads). Each core computes its batch's S x 512 slice of q/k/v, runs
# attention for its 4 heads, and produces a partial [S, E] out-proj
# contribution; the host sums the 4 partials per batch.
import numpy as np

B, S, E, H, HD, ROT, MAXP = 2, 1024, 2048, 16, 128, 64, 2048
HL = 4            # heads per core
GD = HL * HD      # 512 output dims per core
NB = S // 128     # 8 s-blocks
NK = E // 128     # 16 contraction tiles
EPS = 1e-12


def _sinusoidal(num_pos, dim):
    inv_freq = 1.0 / (10000.0 ** (np.arange(0, dim, 2, dtype=np.float32) / dim))
    sinusoid = np.einsum("i,j->ij", np.arange(num_pos, dtype=np.float32), inv_freq)
    return np.concatenate([np.sin(sinusoid), np.cos(sinusoid)], axis=-1)


_BUILT = None


def _build():
    global _BUILT
    if _BUILT is not None:
        return _BUILT
    import concourse.bacc as bacc
    import concourse.mybir as mybir
    from concourse.tile import TileContext

    F32 = mybir.dt.float32
    F32R = mybir.dt.float32r
    BF16 = mybir.dt.bfloat16
    MUL = mybir.AluOpType.mult
    SQUARE = mybir.ActivationFunctionType.Square

    nc = bacc.Bacc(None, target_bir_lowering=False)

    hsT = nc.dram_tensor("hsT", [NB, 128, NK * 128], BF16, kind="ExternalInput")
    wqT = nc.dram_tensor("wqT", [E, GD], BF16, kind="ExternalInput")
    wkT = nc.dram_tensor("wkT", [E, GD], BF16, kind="ExternalInput")
    wvT = nc.dram_tensor("wvT", [E, GD], BF16, kind="ExternalInput")
    woT = nc.dram_tensor("woT", [GD, E], BF16, kind="ExternalInput")
    cos4d = nc.dram_tensor("cos4", [128, NB, HL, ROT], BF16, kind="ExternalInput")
    sin4d = nc.dram_tensor("sin4", [128, NB, HL, ROT], BF16, kind="ExternalInput")
    masksd = nc.dram_tensor("masks", [128, 4, 512], F32R, kind="ExternalInput")
    vscaled = nc.dram_tensor("vscale", [128, NB, HL], F32, kind="ExternalInput")
    qmaskd = nc.dram_tensor("qmask", [128, NB], F32, kind="ExternalInput")
    identd = nc.dram_tensor("ident", [128, 128], BF16, kind="ExternalInput")
    outd = nc.dram_tensor("out", [S, E], BF16, kind="ExternalOutput")

    import concourse.bass as bass
    with TileContext(nc) as tc:
        from contextlib import ExitStack
        ctx = ExitStack()
        with ctx:
            const = ctx.enter_context(tc.tile_pool(name="const", bufs=1))
            qkT_pool = ctx.enter_context(tc.tile_pool(name="qkT", bufs=1))
            vn_pool = ctx.enter_context(tc.tile_pool(name="vn", bufs=1))
            scr = ctx.enter_context(tc.tile_pool(name="scr", bufs=4))
            rot_pool = ctx.enter_context(tc.tile_pool(name="rot", bufs=5))
            ps_proj = ctx.enter_context(tc.tile_pool(name="ps_proj", bufs=3, space="PSUM"))
            ps_tr = ctx.enter_context(tc.tile_pool(name="ps_tr", bufs=1, space="PSUM"))
            ps_at = ctx.enter_context(tc.tile_pool(name="ps_at", bufs=2, space="PSUM"))
            ps_ao = ctx.enter_context(tc.tile_pool(name="ps_ao", bufs=2, space="PSUM"))

            cos4 = const.tile([128, NB, HL, ROT], BF16)
            sin4 = const.tile([128, NB, HL, ROT], BF16)
            masks = const.tile([128, 4, 512], F32R)
            vscale = const.tile([128, NB, HL], F32)
            qmask = const.tile([128, NB], F32)
            ident = const.tile([128, 128], BF16)
            # only the tables needed during Q-proj load up front; the rest
            # (vscale/masks) are issued later from program positions so their
            # transfers don't steal DMA bandwidth from wq/hs
            nc.scalar.dma_start(out=ident[:], in_=identd[:])
            nc.scalar.dma_start(out=qmask[:], in_=qmaskd[:])
            nc.scalar.dma_start(out=cos4[:], in_=cos4d[:])
            nc.scalar.dma_start(out=sin4[:], in_=sin4d[:])

            # HAM warmup: keep PE busy on dummy matmuls over a memset tile
            # (no DMA dependency) so the clock gate opens to 2.4 GHz before
            # the DMA-paced Q-projection starts issuing real matmuls. Sized
            # to bridge the whole wq+hs load window without a >3us PE idle.
            wdum = const.tile([128, 128], BF16, name="wdum")
            nc.vector.memset(wdum[:], 0.0)
            warm_ps = ps_tr.tile([128, 128], F32, tag="pt")
            for _ in range(96):
                nc.tensor.matmul(warm_ps[:], wdum[:], wdum[:],
                                 start=True, stop=True)

            # persistent transposed q/k: per local head, [hd=128, S]
            qT = [qkT_pool.tile([128, S], BF16, name=f"qT{h}") for h in range(HL)]
            kT = [qkT_pool.tile([128, S], BF16, name=f"kT{h}") for h in range(HL)]
            # v in natural layout per s-block: [128, 512]
            vn = [vn_pool.tile([128, GD], BF16, name=f"vn{m}") for m in range(NB)]

            with tc.tile_pool(name="hs", bufs=1) as hs_pool, \
                 tc.tile_pool(name="w", bufs=3) as w_pool:
                # hs blocked per s-block m: [128 E-sub, m, k, 128 s]
                hs = hs_pool.tile([128, NB, NK, 128], BF16)

                # one batched DMA per weight half (DMA *issues* cost
                # ~700ns each on the issuing engine - batch aggressively).
                # wk/wv are issued from the scalar queue at program positions
                # mid-projection, which delays their transfers until the
                # earlier loads have drained (transfers share 16 DMA engines).
                def load_w(wd, eng):
                    wt = w_pool.tile([128, NK, GD], BF16, tag="w")
                    for hf in range(2):
                        eng.dma_start(
                            out=wt[:, hf * (NK // 2):(hf + 1) * (NK // 2)],
                            in_=bass.AP(wd, hf * (NK // 2) * 128 * GD,
                                        [[GD, 128], [128 * GD, NK // 2],
                                         [1, GD]]))
                    return [wt[:, k] for k in range(NK)]

                # wq first (m=0 needs all of it); hs per m-block on the
                # gpsimd queue so m=0 only waits for its own slice
                wq = load_w(wqT, nc.sync)
                for m in range(NB):
                    nc.gpsimd.dma_start(out=hs[:, m], in_=hsT[m])

                def proj_mms(wtiles, m):
                    ps = ps_proj.tile([128, GD], F32)
                    for k in range(NK):
                        nc.tensor.matmul(
                            ps[:], hs[:, m, k],
                            wtiles[k], start=(k == 0), stop=(k == NK - 1))
                    return ps

                def qk_postproc(ps, m):
                    # sum-of-squares per head (rotary is norm-preserving, so
                    # norms can be computed pre-rotary, straight from PSUM)
                    ss = scr.tile([128, HL], F32, tag="ss")
                    sqs = scr.tile([128, 128], F32, tag="sqs", bufs=1)
                    for h in range(HL):
                        nc.scalar.activation(out=sqs[:],
                                             in_=ps[:, h * 128:(h + 1) * 128],
                                             func=SQUARE, accum_out=ss[:, h:h + 1])
                    nrm = scr.tile([128, HL], F32, tag="nrm")
                    nc.scalar.sqrt(nrm[:], ss[:])
                    nc.vector.tensor_scalar_max(nrm[:], nrm[:], EPS)
                    rr = scr.tile([128, HL], F32, tag="rr")
                    nc.vector.reciprocal(rr[:], nrm[:])
                    nc.vector.tensor_scalar_mul(rr[:], rr[:], qmask[:, m:m + 1])
                    # evict PSUM -> SBUF with the per-row scale folded in
                    qn = rot_pool.tile([128, HL, 128], BF16, tag="qn")
                    for h in range(HL):
                        nc.vector.tensor_scalar_mul(qn[:, h], ps[:, h * 128:(h + 1) * 128],
                                                    rr[:, h:h + 1])
                    # GPT-J interleaved rotary on first ROT dims of each head
                    qrot = rot_pool.tile([128, HL, ROT], BF16, tag="qrot", bufs=2)
                    tmp2 = rot_pool.tile([128, HL, ROT], BF16, tag="tmp2", bufs=2)
                    nc.gpsimd.tensor_tensor(out=qrot[:, :, 0:ROT:2], in0=qn[:, :, 1:ROT:2],
                                            in1=sin4[:, m, :, 0:ROT:2], op=MUL)
                    nc.gpsimd.tensor_tensor(out=qrot[:, :, 1:ROT:2], in0=qn[:, :, 0:ROT:2],
                                            in1=sin4[:, m, :, 1:ROT:2], op=MUL)
                    nc.gpsimd.tensor_tensor(out=tmp2[:], in0=qn[:, :, 0:ROT],
                                            in1=cos4[:, m], op=MUL)
                    nc.gpsimd.tensor_add(out=qn[:, :, 0:ROT], in0=qrot[:], in1=tmp2[:])
                    return qn

                def transpose_block(qn, m, dstT):
                    for h in range(HL):
                        pt = ps_tr.tile([128, 128], BF16)
                        nc.tensor.transpose(pt[:], qn[:, h], ident[:])
                        nc.vector.tensor_copy(dstT[h][:, m * 128:(m + 1) * 128], pt[:])

                LAG = 3
                qns = {}
                wk = wv = None
                for m in range(NB):
                    qns[m] = qk_postproc(proj_mms(wq, m), m)
                    if m == 1:   # scalar engine reaches this at ~t(m=1 done)
                        wk = load_w(wkT, nc.scalar)
                    if m >= LAG:
                        transpose_block(qns.pop(m - LAG), m - LAG, qT)
                for m in range(NB - LAG, NB):
                    transpose_block(qns.pop(m), m, qT)
                for m in range(NB):
                    qns[m] = qk_postproc(proj_mms(wk, m), m)
                    if m == 1:
                        wv = load_w(wvT, nc.scalar)
                        nc.scalar.dma_start(out=vscale[:], in_=vscaled[:])
                    if m >= LAG:
                        transpose_block(qns.pop(m - LAG), m - LAG, kT)
                nc.scalar.dma_start(out=masks[:], in_=masksd[:])
                for m in range(NB - LAG, NB):
                    transpose_block(qns.pop(m), m, kT)
                for m in range(NB):
                    ps = proj_mms(wv, m)
                    for h in range(HL):
                        nc.vector.tensor_scalar_mul(vn[m][:, h * 128:(h + 1) * 128],
                                                    ps[:, h * 128:(h + 1) * 128],
                                                    vscale[:, m, h:h + 1])

            with tc.tile_pool(name="atn", bufs=14) as atn_pool, \
                 tc.tile_pool(name="aT", bufs=1) as aT_pool, \
                 tc.tile_pool(name="wo", bufs=1) as wo_pool, \
                 tc.tile_pool(name="ost", bufs=3) as ost_pool:
                aT = [aT_pool.tile([128, S], BF16, name=f"aT{h}") for h in range(HL)]

                # single batched DMA: wo_all[p, kk, :] = woT[kk*128+p, :]
                wo_all = wo_pool.tile([128, 4, E], BF16, tag="wo")
                nc.scalar.dma_start(
                    out=wo_all[:],
                    in_=bass.AP(woT, 0, [[E, 128], [128 * E, 4], [1, E]]))
                wo_tiles = [[wo_all[:, kk, n * 512:(n + 1) * 512]
                             for kk in range(4)] for n in range(4)]

                GE = mybir.AluOpType.is_ge
                diag_rr = 0
                for c in range(2):
                    for h in range(HL):
                        nblk = 4 * (c + 1)
                        at_tiles = []
                        for j in range(nblk):
                            pa = ps_at.tile([128, 512], F32)
                            nc.tensor.matmul(pa[:], kT[h][:, j * 128:(j + 1) * 128],
                                             qT[h][:, c * 512:(c + 1) * 512],
                                             start=True, stop=True)
                            at = atn_pool.tile([128, 512], BF16, tag="at")
                            jj = j - c * 4
                            if jj >= 0:  # diagonal block: apply causal mask
                                nc.vector.tensor_tensor(out=at[:], in0=pa[:],
                                                        in1=masks[:, jj], op=MUL)
                            else:        # fully below the diagonal
                                nc.scalar.copy(at[:], pa[:])
                            at_tiles.append(at)
                        po = ps_ao.tile([128, 512], F32)
                        for j in range(nblk):
                            nc.tensor.matmul(po[:], vn[j][:, h * 128:(h + 1) * 128],
                                             at_tiles[j][:],
                                             start=(j == 0), stop=(j == nblk - 1))
                        nc.scalar.copy(aT[h][:, c * 512:(c + 1) * 512], po[:])
                    # out-proj for this query-half overlaps next half's attn
                    for m in range(c * 4, (c + 1) * 4):
                        ot = ost_pool.tile([128, E], BF16, tag="ot")
                        for n in range(4):
                            ps = ps_proj.tile([128, 512], F32, tag="ps")
                            for k in range(HL):
                                nc.tensor.matmul(ps[:], aT[k][:, m * 128:(m + 1) * 128],
                                                 wo_tiles[n][k],
                                                 start=(k == 0), stop=(k == HL - 1))
                            # Vector carries the diag-mask evicts; bias the
                            # out-proj evictions toward Scalar to balance
                            if (4 * m + n) % 8 < 3:
                                nc.vector.tensor_copy(ot[:, n * 512:(n + 1) * 512], ps[:])
                            else:
                                nc.scalar.copy(ot[:, n * 512:(n + 1) * 512], ps[:])
                        nc.sync.dma_start(out=outd[m * 128:(m + 1) * 128, :], in_=ot[:])

    nc.compile()
    _BUILT = nc
    return nc


def _prep_inputs(hidden_states, w_q, w_k, w_v, w_o, norm_const,
                 attention_mask, position_ids):
    """Host-side shard + table prep. Returns list of 8 in_maps."""
    import ml_dtypes
    BF = ml_dtypes.bfloat16
    hidden_states = np.asarray(hidden_states, dtype=np.float32)
    w_q = np.asarray(w_q, dtype=np.float32)
    w_k = np.asarray(w_k, dtype=np.float32)
    w_v = np.asarray(w_v, dtype=np.float32)
    w_o = np.asarray(w_o, dtype=np.float32)
    norm_const = np.asarray(norm_const, dtype=np.float32).reshape(H)
    attention_mask = np.asarray(attention_mask, dtype=np.float32).reshape(B, S)
    position_ids = np.asarray(position_ids).reshape(B, S).astype(np.int64)

    embed = _sinusoidal(MAXP, ROT)                       # [MAXP, 64]
    sig = 1.0 / (1.0 + np.exp(-norm_const.astype(np.float64)))   # [H]
    mask0 = (attention_mask == 0).astype(np.float32)     # [B, S]
    counts = np.cumsum(mask0, axis=1).astype(np.float32)  # [B, S]
    denom = np.maximum(counts[:, None, :] ** sig[None, :, None], 1.0).astype(np.float32)
    vs_full = mask0[:, None, :] / denom                  # [B, H, S]

    # causal masks for the 4 diagonal-band block offsets
    p = np.arange(128)[:, None]
    f = np.arange(512)[None, :]
    masks = np.stack([(jj * 128 + p <= f) for jj in range(4)]).astype(np.float32)
    masks = np.ascontiguousarray(masks.transpose(1, 0, 2))  # [128, 4, 512]
    ident = np.eye(128, dtype=BF)

    in_maps = []
    for b in range(B):
        sincos = embed[position_ids[b]]                  # [S, 64]
        sin, cos = sincos[:, :ROT // 2], sincos[:, ROT // 2:]
        cosR = np.repeat(cos, 2, axis=1)                 # [S, 64]
        sinS = np.empty((S, ROT), dtype=np.float32)
        sinS[:, 0::2] = -sin
        sinS[:, 1::2] = sin
        # [S,64] -> [128 part, NB, 64] -> broadcast over HL heads
        def to4(t):
            t = t.reshape(NB, 128, ROT).transpose(1, 0, 2)
            return np.ascontiguousarray(np.broadcast_to(
                t[:, :, None, :], (128, NB, HL, ROT)).astype(BF))
        cos4 = to4(cosR)
        sin4 = to4(sinS)
        qm = np.ascontiguousarray(mask0[b].reshape(NB, 128).T)  # [128, NB]
        # [NB, 128 E-sub, NK*128]: hsT_b[m, p, k*128+c] = hs[b, m*128+c, k*128+p]
        hsT_b = np.ascontiguousarray(
            hidden_states[b].reshape(NB, 128, NK, 128)
            .transpose(0, 3, 2, 1).astype(BF)).reshape(NB, 128, NK * 128)
        for g in range(4):
            sl = slice(g * GD, (g + 1) * GD)
            vs = vs_full[b, 4 * g:4 * g + HL, :]                # [HL, S]
            vs = np.ascontiguousarray(
                vs.reshape(HL, NB, 128).transpose(2, 1, 0))     # [128, NB, HL]
            in_maps.append({
                "hsT": hsT_b,
                "wqT": np.ascontiguousarray(w_q[sl, :].T.astype(BF)),
                "wkT": np.ascontiguousarray(w_k[sl, :].T.astype(BF)),
                "wvT": np.ascontiguousarray(w_v[sl, :].T.astype(BF)),
                "woT": np.ascontiguousarray(w_o[:, sl].T.astype(BF)),
                "cos4": cos4, "sin4": sin4, "masks": masks,
                "vscale": vs, "qmask": qm, "ident": ident,
            })
    # core order: c = b*4 + g
    return in_maps


def run(inputs, trace=False, trace_cores=None):
    from concourse.bass_utils import run_bass_kernel_spmd
    nc = _build()
    in_maps = _prep_inputs(**inputs)
    res = run_bass_kernel_spmd(nc, in_maps, core_ids=list(range(8)),
                               trace=trace, trace_cores=trace_cores)
    partials = [np.asarray(res.results[c]["out"], dtype=np.float32)
                for c in range(8)]
    out = np.empty((B, S, E), dtype=np.float32)
    for b in range(B):
        out[b] = partials[4 * b] + partials[4 * b + 1] \
            + partials[4 * b + 2] + partials[4 * b + 3]
    return out, res


def kernel(**inputs):
    out, _ = run(inputs, trace=False)
    return out

